# revision 39
# baseline (speedup 1.0000x reference)
"""CapsuleConv2d (k-means routing, 3 iters) Trainium2 Bass kernel.

Problem (hardcoded): x [2,128,32,32] f32, weight [16,16,16,3,3] f32
(w[o,l,m,i,j]), stride 1, pad 1, G=8 groups of M=16 in-channels,
N_in = G*KH*KW = 72 votes, O=16 out-capsules of L=16.
Output [2, 256, 32, 32] f32.

Sharding: data-parallel over (b, oh): 64 rows -> 8 cores x 8 rows.
Each core processes 2 chunks of 128 positions (4 oh-rows x 32 ow).

v2 changes over the original baseline (290us):
 - Host supplies 3 column-shifted slabs (one per kw tap j), so every
   3x3-tap window is a contiguous [*, 128] lhsT slice: the Pool/ACT
   patch-assembly copies are gone and matmuls read the slab directly.
 - The Pool engine runs the full routing chain (z1 / l-tree / z2 /
   n-tree) for votes n in [57, 72) concurrently with DVE's n in
   [0, 57): per-iteration wall time drops from ~37.6us (DVE alone at
   2x bf16) to ~31us (DVE ~30us || Pool ~30us), merged by one add.
 - u PSUM->SBUF drains are split across DVE/ACT/Pool at the head
   (chunk 0) and ride on ACT afterwards.
"""
from contextlib import ExitStack

import numpy as np

B, CIN, H, W = 2, 128, 32, 32
G, M, O, L = 8, 16, 16, 16
NTAP, NIN = 9, 72
COUT = O * L
NCORES = 8
ROWS_PER_CORE = 8  # (b, oh) rows per core
CHUNK_ROWS = 4
NCHUNK = ROWS_PER_CORE // CHUNK_ROWS
P = 128
ND = 57            # votes routed on DVE
NP = NIN - ND      # votes routed on Pool (15)
NH0, NH1 = 29, 28  # DVE n-halves for the exp pipeline


def _build_bass():
    import concourse.tile as tile
    from concourse import bacc, masks, mybir

    # The act-table pass greedily picks the first set containing each
    # function, ping-ponging exp_and_others <-> natural_log (2.7us/load).
    # Strip Exp/Ln from every set except the combined one so all our ACT
    # work (Exp, Ln, Copy, Identity) lives in a single table set.
    if not getattr(bacc, "_capsule_act_tables_patched", False):
        _orig_gat = bacc.get_activation_tables

        def _gat(arch):
            t = dict(_orig_gat(arch))
            for name, fns in t.items():
                if name != "natural_log_exp_and_others":
                    t[name] = {f for f in fns if f.name not in ("Exp", "Ln")}
            return t

        bacc.get_activation_tables = _gat
        bacc._capsule_act_tables_patched = True

    fp32 = mybir.dt.float32
    f32r = mybir.dt.float32r
    bf16 = mybir.dt.bfloat16
    AX = mybir.AxisListType
    AF = mybir.ActivationFunctionType

    nc = bacc.Bacc("TRN2", target_bir_lowering=False, debug=False)
    # f32r params: same bits as fp32, but lets every DMA queue load them
    # without the gpsimd-only cast path.
    # j-shifted m-major slabs: xs[m, j, g, h(10), w(32)]; window for tap
    # (i, j) rows r..r+3 is contiguous 128 floats -> direct matmul lhsT.
    xs_d = nc.declare_dram_parameter("xs", [M, 3 * G * 10 * 32], f32r,
                                     isOutput=False)
    # j-shifted channel-major slabs: xs2[c=(g,m), j, h(10), w(32)]
    xs2_d = nc.declare_dram_parameter("xs2", [CIN, 3 * 10 * 32], f32r,
                                      isOutput=False)
    # host-pretransposed weights, replicated over g on the host so one DMA
    # fills the full K=128 operand: wr2[(g,m), (t,l,o)] = w[o, l, m, i, j]
    w_d = nc.declare_dram_parameter("wgt", [CIN, NTAP * 256], f32r,
                                    isOutput=False)
    out_d = nc.declare_dram_parameter("out", [COUT, ROWS_PER_CORE, W], fp32,
                                      isOutput=True)

    with tile.TileContext(nc) as tc, ExitStack() as ctx:
        const_pool = ctx.enter_context(tc.tile_pool(name="const", bufs=1))
        upool = ctx.enter_context(tc.tile_pool(name="u", bufs=2))
        zpool = ctx.enter_context(tc.tile_pool(name="z", bufs=1))
        lepool = ctx.enter_context(tc.tile_pool(name="le", bufs=1))
        small = ctx.enter_context(tc.tile_pool(name="small", bufs=2))
        psum = ctx.enter_context(tc.tile_pool(name="ps", bufs=3, space="PSUM"))
        tpsum = ctx.enter_context(tc.tile_pool(name="tps", bufs=1, space="PSUM"))
        vpsum = ctx.enter_context(tc.tile_pool(name="vps", bufs=1, space="PSUM"))

        # ---- constants (once per core) ----
        # Exactly 4 input DMAs (HWDGE descriptor-gen is ~625ns serial per
        # DMA): slab first (gates every u-matmul), then wgt in partition
        # halves (a single 128-partition DMA costs ~3.3us, halves ~0.5us
        # each), slab2 for the K=128 v0.
        wr2 = const_pool.tile([CIN, NTAP * 256], f32r)
        slab_f = const_pool.tile([M, 3 * G * 10 * 32], f32r)
        slab = slab_f[:].rearrange("m (j g h w) -> m j g h w", j=3, g=G, h=10)
        nc.sync.dma_start(out=slab_f[:], in_=xs_d[:])
        nc.scalar.dma_start(out=wr2[0:64, :], in_=w_d[0:64, :])
        slab2_f = const_pool.tile([CIN, 3 * 10 * 32], f32r)
        slab2 = slab2_f[:].rearrange("c (j h w) -> c j h w", j=3, h=10)
        nc.gpsimd.dma_start(out=slab2_f[:], in_=xs2_d[:])
        nc.sync.dma_start(out=wr2[64:128, :], in_=w_d[64:128, :])
        wr_r = wr2[0:M, :]

        ident = const_pool.tile([128, 128], fp32)
        masks.make_identity(nc, ident[:])

        # PE warm-up: ~4us of back-to-back dummy matmuls during the initial
        # DMA wait releases the HAM clock throttle before the real matmuls.
        warm = const_pool.tile([128, 64], bf16)
        nc.vector.memset(warm[:], 0.0)
        wps = tpsum.tile([64, 64], fp32, tag="tp")
        for _ in range(55):
            nc.tensor.matmul(wps[:], warm[:, 0:64], warm[:], start=True,
                             stop=True, tile_position=(0, 0))

        def priors_v0(k):
            # ---- v0 = sum_n u via K=128 matmuls; finishes early so prep
            # (hoisted before the u fill) can compute vn immediately.
            v0_ps = vpsum.tile([P, 256], fp32, tag="v0ps")
            for t in range(NTAP):
                i, j = divmod(t, 3)
                lhsT2 = slab2[:, j, 4 * k + i: 4 * k + i + CHUNK_ROWS, :]
                nc.tensor.matmul(v0_ps[:], lhsT2,
                                 wr2[:, t * 256:(t + 1) * 256],
                                 start=(t == 0), stop=(t == NTAP - 1),
                                 tile_position=(0, 0), skip_group_check=True)
            v_u = small.tile([P, 256], fp32, tag="vu")
            with tc.high_priority():
                nc.scalar.copy(v_u[:], v0_ps[:])
            S = small.tile([P, O], fp32, tag="S")
            u_sb = upool.tile([P, NIN * 256], bf16, tag="u")
            return {"v_u": v_u, "S": S, "vn": None, "k": k, "u_sb": u_sb}

        # u-slot drain groups: Pool's slice [ND, NIN) fills and drains
        # first so the Pool routing chain starts on time, then DVE's h0 and
        # h1. 'D'/'A' pick the drain engine (chunk 0 splits DVE/ACT while
        # DVE is otherwise idle; chunk 1 rides entirely on ACT slack).
        # u-slot drain groups: Pool's slice [ND, NIN) fills and drains
        # first so the Pool routing chain starts on time, then DVE's h0 and
        # h1. 'D'/'A' pick the drain engine; 'D' groups also give DVE
        # useful work while it waits on the vn/u joins.
        GROUPS_P1 = [(57, 61, 'A'), (61, 65, 'D'), (65, 69, 'A'),
                     (69, 72, 'D'),
                     (0, 4, 'A'), (4, 8, 'D'), (8, 12, 'A'), (12, 16, 'D'),
                     (16, 20, 'A'), (20, 24, 'D'), (24, 28, 'A'),
                     (28, 29, 'D')]
        GROUPS_P2 = [(29, 33, 'A'), (33, 37, 'D'), (37, 41, 'A'),
                     (41, 45, 'A'), (45, 49, 'D'), (49, 53, 'A'),
                     (53, 57, 'A')]

        def priors_u(st, groups):
            # ---- priors on PE (fp32r: full-rate, near-fp32 precision).
            # Slot s holds vote (t, g) = divmod(s, 8); routing is symmetric
            # over votes so the assignment is free.
            k = st["k"]
            u_sb = st["u_sb"]
            for (s0, s1, ec) in groups:
                ups = psum.tile([P, 1024], fp32, tag="ups")
                for s in range(s0, s1):
                    t, g = divmod(s, 8)
                    i, j = divmod(t, 3)
                    lhsT = slab[:, j, g, 4 * k + i: 4 * k + i + CHUNK_ROWS, :]
                    nc.tensor.matmul(
                        ups[:, (s - s0) * 256:(s - s0 + 1) * 256],
                        lhsT, wr_r[:, t * 256:(t + 1) * 256],
                        start=True, stop=True, tile_position=(0, 0))
                dst = u_sb[:, s0 * 256:s1 * 256]
                src = ups[:, 0:(s1 - s0) * 256]
                if ec == 'D':
                    nc.vector.tensor_copy(dst, src)
                else:
                    nc.scalar.copy(dst, src)

        def prep(st):
            # vn = v_u * rsqrt(max(sum_l v_u^2, eps)); rsqrt = exp(-ln/2)
            # High priority end to end: vn gates BOTH engines' next
            # iteration, and at normal priority the scheduler parks this
            # chain behind a whole routing slice (Pool then starves ~13us).
            v_u = st["v_u"]
            sq = small.tile([P, 256], fp32, tag="sq")
            w2 = small.tile([P, O], fp32, tag="w2")
            lg = small.tile([P, O], fp32, tag="lg")
            rn = small.tile([P, O], fp32, tag="rn")
            vn = small.tile([P, 256], bf16, tag="vn")
            with tc.high_priority():
                nc.vector.tensor_mul(sq[:], v_u[:], v_u[:])
                nc.vector.reduce_sum(
                    w2[:], sq[:].rearrange("p (l o) -> p o l", l=L), axis=AX.X)
                nc.vector.tensor_scalar_max(w2[:], w2[:], 1e-24)
                nc.scalar.activation(lg[:], w2[:], AF.Ln)
                nc.scalar.activation(rn[:], lg[:], AF.Exp, scale=-0.5)
                nc.vector.tensor_mul(
                    vn[:].rearrange("p (l o) -> p l o", l=L),
                    v_u[:].rearrange("p (l o) -> p l o", l=L),
                    rn[:].unsqueeze(1).broadcast_to([P, L, O]))
            st["vn"] = vn

        def main(st, it):
            u_sb, vn, S = st["u_sb"], st["vn"], st["S"]
            vn_lo = vn[:].rearrange("p (l o) -> p l o", l=L)
            # DVE z region [0 : ND*256] + tree scratch (tr: max(l-tree h0
            # NH0*160, n-tree L1 28*256) = 7424; tr2: l-tree h1 NH1*160)
            TRW = NH0 * 256  # 7424
            zt = zpool.tile([P, ND * 256 + TRW + NH1 * 160], bf16, tag="z")
            z = zt[:, 0:ND * 256]
            tr = zt[:, ND * 256: ND * 256 + TRW]
            tr2 = zt[:, ND * 256 + TRW:]
            # Pool z region (ptr covers the l-tree's NP*160 = 2400 elems)
            pzt = zpool.tile([P, NP * 256 + 2400 + 512], bf16, tag="pz")
            pz = pzt[:, 0:NP * 256]
            ptr = pzt[:, NP * 256: NP * 256 + 2400]
            ptr2 = pzt[:, NP * 256 + 2400:]

            # logits/e lifetime is within this main call (mains execute
            # in-order per engine), so single-buffered is safe.
            logits = lepool.tile([P, NIN * O], fp32, tag="logits")
            e = lepool.tile([P, NIN * O], bf16, tag="e")

            def zv(ap, lw, nn):
                return ap.rearrange("p (n l o) -> p n l o", n=nn, l=lw)

            def half(eng, zh, uh, trh, lgh, nh):
                # z1 = u * bcast_n(vn); l-tree; logits slice
                eng.tensor_mul(
                    zv(zh, L, nh), uh,
                    vn_lo.unsqueeze(1).broadcast_to([P, nh, L, O]))
                eng.tensor_add(zv(trh[:, 0:nh * 128], 8, nh),
                               zv(zh, L, nh)[:, :, 0:8, :],
                               zv(zh, L, nh)[:, :, 8:16, :])
                eng.tensor_add(zv(zh[:, 0:nh * 64], 4, nh),
                               zv(trh[:, 0:nh * 128], 8, nh)[:, :, 0:4, :],
                               zv(trh[:, 0:nh * 128], 8, nh)[:, :, 4:8, :])
                eng.tensor_add(zv(trh[:, nh * 128:nh * 160], 2, nh),
                               zv(zh[:, 0:nh * 64], 4, nh)[:, :, 0:2, :],
                               zv(zh[:, 0:nh * 64], 4, nh)[:, :, 2:4, :])
                t2 = zv(trh[:, nh * 128:nh * 160], 2, nh)
                eng.tensor_add(lgh.rearrange("p (n o) -> p n o", n=nh),
                               t2[:, :, 0, :], t2[:, :, 1, :])

            def z2(eng, zh, uh, eh, nh):
                eng.tensor_mul(
                    zv(zh, L, nh), uh,
                    eh.rearrange("p (n o) -> p n o", n=nh)
                    .unsqueeze(2).broadcast_to([P, nh, L, O]))

            # ---- DVE slice: two n-halves pipelined through the ACT exp ----
            offs = [(0, NH0), (NH0, NH1)]
            for (n0, nh) in offs:
                zh = z[:, n0 * 256:(n0 + nh) * 256]
                uh = zv(u_sb[:, n0 * 256:(n0 + nh) * 256], L, nh)
                trh = tr if n0 == 0 else tr2
                lgh = logits[:, n0 * O:(n0 + nh) * O]
                half(nc.vector, zh, uh, trh, lgh, nh)
                with tc.high_priority():
                    nc.scalar.activation(e[:, n0 * O:(n0 + nh) * O], lgh,
                                         AF.Exp)
                z2(nc.vector, zh, uh, e[:, n0 * O:(n0 + nh) * O], nh)

            # ---- Pool slice: same chain for n in [ND, NIN) ----
            pzh = pz
            puh = zv(u_sb[:, ND * 256:], L, NP)
            plg = logits[:, ND * O:]
            half(nc.gpsimd, pzh, puh, ptr, plg, NP)
            with tc.high_priority():
                nc.scalar.activation(e[:, ND * O:], plg, AF.Exp)
            z2(nc.gpsimd, pzh, puh, e[:, ND * O:], NP)

            if it == 2:
                nc.vector.reduce_sum(
                    S[:], e[:].rearrange("p (n o) -> p o n", n=NIN),
                    axis=AX.X)

            def add(eng, out, a, b):
                eng.tensor_add(out, a, b)

            # ---- DVE n-tree: 57 -> 28(+1) -> 14(+1c) ... leftovers merged
            # at the end. Block row = 256 elems.
            A = nc.vector
            add(A, tr[:, 0:28 * 256], z[:, 0:28 * 256], z[:, 28 * 256:56 * 256])
            # leftover: z row 56
            add(A, z[:, 0:14 * 256], tr[:, 0:14 * 256], tr[:, 14 * 256:28 * 256])
            add(A, tr[:, 0:7 * 256], z[:, 0:7 * 256], z[:, 7 * 256:14 * 256])
            add(A, z[:, 0:3 * 256], tr[:, 0:3 * 256], tr[:, 3 * 256:6 * 256])
            # leftover: tr row 6
            add(A, tr2[:, 0:256], z[:, 0:256], z[:, 256:512])
            # rows left: tr2[0], z row2, tr row6, z row56
            add(A, z[:, 256:512], z[:, 2 * 256:3 * 256], z[:, 56 * 256:57 * 256])
            add(A, z[:, 0:256], tr2[:, 0:256], tr[:, 6 * 256:7 * 256])
            v_d = small.tile([P, 256], fp32, tag="vd")
            add(A, v_d[:], z[:, 0:256], z[:, 256:512])

            # ---- Pool n-tree: 15 -> 7(+1) -> 3(+1c) -> 1(+1c)
            Pp = nc.gpsimd
            add(Pp, ptr[:, 0:7 * 256], pz[:, 0:7 * 256], pz[:, 7 * 256:14 * 256])
            # leftover pz row 14
            add(Pp, pz[:, 0:3 * 256], ptr[:, 0:3 * 256], ptr[:, 3 * 256:6 * 256])
            # leftover ptr row 6
            add(Pp, ptr2[:, 0:256], pz[:, 0:256], pz[:, 256:512])
            add(Pp, pz[:, 0:256], pz[:, 2 * 256:3 * 256], pz[:, 14 * 256:15 * 256])
            add(Pp, ptr2[:, 256:512], ptr2[:, 0:256], ptr[:, 6 * 256:7 * 256])
            v_p = small.tile([P, 256], fp32, tag="vp")
            add(Pp, v_p[:], pz[:, 0:256], ptr2[:, 256:512])

            # merge (high priority: first link of the vn chain)
            v_u = small.tile([P, 256], fp32, tag="vu")
            with tc.high_priority():
                nc.vector.tensor_add(v_u[:], v_d[:], v_p[:])
            st["v_u"] = v_u

        def squash(st, k):
            # ---- squash: out = v_u * sqrt(w2) / (S^2 + w2) ----
            # Pure serial chain at the very end: keep it on DVE+ACT (fewer
            # cross-engine hops than spreading it over Pool).
            v_u, S = st["v_u"], st["S"]
            sq = small.tile([P, 256], fp32, tag="sq")
            nc.vector.tensor_mul(sq[:], v_u[:], v_u[:])
            w2 = small.tile([P, O], fp32, tag="w2")
            nc.vector.reduce_sum(
                w2[:], sq[:].rearrange("p (l o) -> p o l", l=L), axis=AX.X)
            nc.vector.tensor_scalar_max(w2[:], w2[:], 1e-24)
            lg = small.tile([P, O], fp32, tag="lg")
            nc.scalar.activation(lg[:], w2[:], AF.Ln)
            sw = small.tile([P, O], fp32, tag="sw")
            nc.scalar.activation(sw[:], lg[:], AF.Exp, scale=0.5)
            den = small.tile([P, O], fp32, tag="den")
            nc.vector.tensor_mul(den[:], S[:], S[:])
            nc.vector.tensor_add(den[:], den[:], w2[:])
            rden = small.tile([P, O], fp32, tag="rn")
            nc.vector.reciprocal(rden[:], den[:])
            fac = small.tile([P, O], fp32, tag="fac")
            nc.vector.tensor_mul(fac[:], sw[:], rden[:])
            # vfin [p, (o,l)] = v_u viewed (o,l) * bcast_l(fac)
            vfin = small.tile([P, 256], fp32, tag="vfin")
            nc.vector.tensor_mul(
                vfin[:].rearrange("p (o l) -> p o l", o=O),
                v_u[:].rearrange("p (l o) -> p o l", l=L),
                fac[:].unsqueeze(2).broadcast_to([P, O, L]))
            # transpose to channel-major and store
            for half_i in range(2):
                tp = tpsum.tile([128, 128], fp32, tag="tp")
                nc.tensor.transpose(tp[:],
                                    vfin[:, half_i * 128:(half_i + 1) * 128],
                                    ident[:])
                vT = small.tile([128, 128], fp32, tag="vT")
                nc.scalar.copy(vT[:], tp[:])
                nc.sync.dma_start(
                    out=out_d[half_i * 128:(half_i + 1) * 128,
                              4 * k:4 * k + CHUNK_ROWS, :],
                    in_=vT[:].rearrange("f (r w) -> f r w", r=CHUNK_ROWS))

        # Interleave the two chunks' routing iterations: chunk k's small
        # ACT chains (prep/exp) overlap the other chunk's DVE/Pool work.
        st0 = priors_v0(0)
        priors_u(st0, GROUPS_P1)
        prep(st0)
        priors_u(st0, GROUPS_P2)
        main(st0, 0)
        prep(st0)
        st1 = priors_v0(1)
        priors_u(st1, GROUPS_P1)
        prep(st1)
        priors_u(st1, GROUPS_P2)
        sts = [st0, st1]
        main(st1, 0)
        prep(st1)
        for k in range(NCHUNK):
            main(sts[k], 1)
            prep(sts[k])
        for k in range(NCHUNK):
            main(sts[k], 2)
            squash(sts[k], k)
    nc.compile()
    return nc


_NC_CACHE = {}


def _get_nc():
    if "nc" not in _NC_CACHE:
        _NC_CACHE["nc"] = _build_bass()
    return _NC_CACHE["nc"]


def _shard_inputs(x, weight):
    # wr[m, (t, l, o)] = weight[o, l, m, i, j], t = i*3+j; replicated over
    # g on the host so one DMA fills the K=128 operand.
    wr = np.ascontiguousarray(np.tile(
        weight.transpose(2, 3, 4, 1, 0).reshape(M, NTAP * 256)
        .astype(np.float32), (G, 1)))
    in_maps = []
    for core in range(NCORES):
        b = core // 4
        oh0 = (core % 4) * ROWS_PER_CORE
        xs = np.zeros((CIN, 10, 34), np.float32)
        lo, hi = oh0 - 1, oh0 + 9
        vlo, vhi = max(lo, 0), min(hi, H)
        xs[:, vlo - lo:vhi - lo, 1:33] = x[b, :, vlo:vhi, :]
        # j-shifted channel-major: xs2j[c, j, h, 32]
        xs2j = np.stack([xs[:, :, j:j + 32] for j in range(3)], axis=1)
        xs2j = np.ascontiguousarray(xs2j.reshape(CIN, 3 * 10 * 32))
        # j-shifted m-major: xsj[m, j, g, h, 32]
        xs_m = xs.reshape(G, M, 10, 34)
        xsj = np.stack([xs_m[:, :, :, j:j + 32] for j in range(3)], axis=2)
        # [g, m, j, h, w] -> [m, j, g, h, w]
        xsj = np.ascontiguousarray(
            xsj.transpose(1, 2, 0, 3, 4).reshape(M, 3 * G * 10 * 32))
        in_maps.append({"xs": xsj, "xs2": xs2j, "wgt": wr})
    return in_maps


def _gather_output(results):
    out = np.zeros((B, COUT, H, W), np.float32)
    for core in range(NCORES):
        b = core // 4
        oh0 = (core % 4) * ROWS_PER_CORE
        out[b, :, oh0:oh0 + ROWS_PER_CORE, :] = results[core]["out"]
    return out


def kernel(x: np.ndarray, weight: np.ndarray) -> np.ndarray:
    from concourse.bass_utils import run_bass_kernel_spmd

    x = np.asarray(x, np.float32)
    weight = np.asarray(weight, np.float32)
    res = run_bass_kernel_spmd(_get_nc(), _shard_inputs(x, weight),
                               list(range(NCORES)))
    return _gather_output(res.results)


# revision 50
# speedup vs baseline: 1.0158x; 1.0158x over previous
"""CapsuleConv2d (k-means routing, 3 iters) Trainium2 Bass kernel.

Problem (hardcoded): x [2,128,32,32] f32, weight [16,16,16,3,3] f32
(w[o,l,m,i,j]), stride 1, pad 1, G=8 groups of M=16 in-channels,
N_in = G*KH*KW = 72 votes, O=16 out-capsules of L=16.
Output [2, 256, 32, 32] f32.

Sharding: data-parallel over (b, oh): 64 rows -> 8 cores x 8 rows.
Each core processes 2 chunks of 128 positions (4 oh-rows x 32 ow).

v2 changes over the original baseline (290us):
 - Host supplies 3 column-shifted slabs (one per kw tap j), so every
   3x3-tap window is a contiguous [*, 128] lhsT slice: the Pool/ACT
   patch-assembly copies are gone and matmuls read the slab directly.
 - The Pool engine runs the full routing chain (z1 / l-tree / z2 /
   n-tree) for votes n in [57, 72) concurrently with DVE's n in
   [0, 57): per-iteration wall time drops from ~37.6us (DVE alone at
   2x bf16) to ~31us (DVE ~30us || Pool ~30us), merged by one add.
 - u PSUM->SBUF drains are split across DVE/ACT/Pool at the head
   (chunk 0) and ride on ACT afterwards.
"""
from contextlib import ExitStack

import numpy as np

B, CIN, H, W = 2, 128, 32, 32
G, M, O, L = 8, 16, 16, 16
NTAP, NIN = 9, 72
COUT = O * L
NCORES = 8
ROWS_PER_CORE = 8  # (b, oh) rows per core
CHUNK_ROWS = 4
NCHUNK = ROWS_PER_CORE // CHUNK_ROWS
P = 128
ND = 57            # votes routed on DVE
NP = NIN - ND      # votes routed on Pool (15)
NH0, NH1 = 29, 28  # DVE n-halves for the exp pipeline


def _build_bass():
    import concourse.tile as tile
    from concourse import bacc, masks, mybir

    # The act-table pass greedily picks the first set containing each
    # function, ping-ponging exp_and_others <-> natural_log (2.7us/load).
    # Strip Exp/Ln from every set except the combined one so all our ACT
    # work (Exp, Ln, Copy, Identity) lives in a single table set.
    if not getattr(bacc, "_capsule_act_tables_patched", False):
        _orig_gat = bacc.get_activation_tables

        def _gat(arch):
            t = dict(_orig_gat(arch))
            for name, fns in t.items():
                if name != "natural_log_exp_and_others":
                    t[name] = {f for f in fns if f.name not in ("Exp", "Ln")}
            return t

        bacc.get_activation_tables = _gat
        bacc._capsule_act_tables_patched = True

    fp32 = mybir.dt.float32
    f32r = mybir.dt.float32r
    bf16 = mybir.dt.bfloat16
    AX = mybir.AxisListType
    AF = mybir.ActivationFunctionType

    nc = bacc.Bacc("TRN2", target_bir_lowering=False, debug=False)
    # f32r params: same bits as fp32, but lets every DMA queue load them
    # without the gpsimd-only cast path.
    # j-shifted m-major slabs: xs[m, j, g, h(10), w(32)]; window for tap
    # (i, j) rows r..r+3 is contiguous 128 floats -> direct matmul lhsT.
    xs_d = nc.declare_dram_parameter("xs", [M, 3 * G * 10 * 32], f32r,
                                     isOutput=False)
    # j-shifted channel-major slabs: xs2[c=(g,m), j, h(10), w(32)]
    xs2_d = nc.declare_dram_parameter("xs2", [CIN, 3 * 10 * 32], f32r,
                                      isOutput=False)
    # host-pretransposed weights, replicated over g on the host so one DMA
    # fills the full K=128 operand: wr2[(g,m), (t,l,o)] = w[o, l, m, i, j]
    w_d = nc.declare_dram_parameter("wgt", [CIN, NTAP * 256], f32r,
                                    isOutput=False)
    out_d = nc.declare_dram_parameter("out", [COUT, ROWS_PER_CORE, W], fp32,
                                      isOutput=True)

    with tile.TileContext(nc) as tc, ExitStack() as ctx:
        const_pool = ctx.enter_context(tc.tile_pool(name="const", bufs=1))
        upool = ctx.enter_context(tc.tile_pool(name="u", bufs=2))
        zpool = ctx.enter_context(tc.tile_pool(name="z", bufs=1))
        lepool = ctx.enter_context(tc.tile_pool(name="le", bufs=1))
        small = ctx.enter_context(tc.tile_pool(name="small", bufs=2))
        psum = ctx.enter_context(tc.tile_pool(name="ps", bufs=3, space="PSUM"))
        tpsum = ctx.enter_context(tc.tile_pool(name="tps", bufs=1, space="PSUM"))
        vpsum = ctx.enter_context(tc.tile_pool(name="vps", bufs=1, space="PSUM"))

        # ---- constants (once per core) ----
        # Exactly 4 input DMAs (HWDGE descriptor-gen is ~625ns serial per
        # DMA): slab first (gates every u-matmul), then wgt in partition
        # halves (a single 128-partition DMA costs ~3.3us, halves ~0.5us
        # each), slab2 for the K=128 v0.
        wr2 = const_pool.tile([CIN, NTAP * 256], f32r)
        slab_f = const_pool.tile([M, 3 * G * 10 * 32], f32r)
        slab = slab_f[:].rearrange("m (j g h w) -> m j g h w", j=3, g=G, h=10)
        nc.sync.dma_start(out=slab_f[:], in_=xs_d[:])
        nc.scalar.dma_start(out=wr2[0:64, :], in_=w_d[0:64, :])
        slab2_f = const_pool.tile([CIN, 3 * 10 * 32], f32r)
        slab2 = slab2_f[:].rearrange("c (j h w) -> c j h w", j=3, h=10)
        nc.gpsimd.dma_start(out=slab2_f[:], in_=xs2_d[:])
        nc.sync.dma_start(out=wr2[64:128, :], in_=w_d[64:128, :])
        wr_r = wr2[0:M, :]

        ident = const_pool.tile([128, 128], fp32)
        masks.make_identity(nc, ident[:])

        # PE warm-up: ~4us of back-to-back dummy matmuls during the initial
        # DMA wait releases the HAM clock throttle before the real matmuls.
        warm = const_pool.tile([128, 64], bf16)
        nc.vector.memset(warm[:], 0.0)
        wps = tpsum.tile([64, 64], fp32, tag="tp")
        for _ in range(55):
            nc.tensor.matmul(wps[:], warm[:, 0:64], warm[:], start=True,
                             stop=True, tile_position=(0, 0))

        def priors_v0(k):
            # ---- v0 = sum_n u via K=128 matmuls; finishes early so prep
            # (hoisted before the u fill) can compute vn immediately.
            v0_ps = vpsum.tile([P, 256], fp32, tag="v0ps")
            for t in range(NTAP):
                i, j = divmod(t, 3)
                lhsT2 = slab2[:, j, 4 * k + i: 4 * k + i + CHUNK_ROWS, :]
                nc.tensor.matmul(v0_ps[:], lhsT2,
                                 wr2[:, t * 256:(t + 1) * 256],
                                 start=(t == 0), stop=(t == NTAP - 1),
                                 tile_position=(0, 0), skip_group_check=True)
            v_u = small.tile([P, 256], fp32, tag="vu")
            with tc.high_priority():
                nc.scalar.copy(v_u[:], v0_ps[:])
            S = small.tile([P, O], fp32, tag="S")
            u_sb = upool.tile([P, NIN * 256], bf16, tag="u")
            return {"v_u": v_u, "S": S, "vn": None, "k": k, "u_sb": u_sb}

        # u-slot drain groups: Pool's slice [ND, NIN) fills and drains
        # first so the Pool routing chain starts on time, then DVE's h0 and
        # h1. 'D'/'A' pick the drain engine (chunk 0 splits DVE/ACT while
        # DVE is otherwise idle; chunk 1 rides entirely on ACT slack).
        # u-slot drain groups: Pool's slice [ND, NIN) fills and drains
        # first so the Pool routing chain starts on time, then DVE's h0 and
        # h1. 'D'/'A' pick the drain engine; 'D' groups also give DVE
        # useful work while it waits on the vn/u joins.
        GROUPS_P1 = [(57, 61, 'A'), (61, 65, 'D'), (65, 69, 'A'),
                     (69, 72, 'D'),
                     (0, 4, 'A'), (4, 8, 'D'), (8, 12, 'A'), (12, 16, 'D'),
                     (16, 20, 'A'), (20, 24, 'D'), (24, 28, 'A'),
                     (28, 29, 'D')]
        GROUPS_P2 = [(29, 33, 'A'), (33, 37, 'D'), (37, 41, 'A'),
                     (41, 45, 'A'), (45, 49, 'D'), (49, 53, 'A'),
                     (53, 57, 'A')]

        def priors_u(st, groups):
            # ---- priors on PE (fp32r: full-rate, near-fp32 precision).
            # Slot s holds vote (t, g) = divmod(s, 8); routing is symmetric
            # over votes so the assignment is free.
            k = st["k"]
            u_sb = st["u_sb"]
            for (s0, s1, ec) in groups:
                ups = psum.tile([P, 1024], fp32, tag="ups")
                for s in range(s0, s1):
                    t, g = divmod(s, 8)
                    i, j = divmod(t, 3)
                    lhsT = slab[:, j, g, 4 * k + i: 4 * k + i + CHUNK_ROWS, :]
                    nc.tensor.matmul(
                        ups[:, (s - s0) * 256:(s - s0 + 1) * 256],
                        lhsT, wr_r[:, t * 256:(t + 1) * 256],
                        start=True, stop=True, tile_position=(0, 0))
                dst = u_sb[:, s0 * 256:s1 * 256]
                src = ups[:, 0:(s1 - s0) * 256]
                if ec == 'D':
                    nc.vector.tensor_copy(dst, src)
                else:
                    nc.scalar.copy(dst, src)

        def prep(st, hoist=False):
            # vn = v_u * rsqrt(max(sum_l v_u^2, eps)); rsqrt = exp(-ln/2)
            # hoist=True runs the chain at high priority so the scheduler
            # slots it into drain gaps (vn gates BOTH engines' next
            # iteration); used where it would otherwise park behind a
            # whole routing slice.
            v_u = st["v_u"]
            sq = small.tile([P, 256], fp32, tag="sq")
            w2 = small.tile([P, O], fp32, tag="w2")
            lg = small.tile([P, O], fp32, tag="lg")
            rn = small.tile([P, O], fp32, tag="rn")
            vn = small.tile([P, 256], bf16, tag="vn")
            nc.vector.tensor_mul(sq[:], v_u[:], v_u[:])
            nc.vector.reduce_sum(
                w2[:], sq[:].rearrange("p (l o) -> p o l", l=L), axis=AX.X)
            nc.vector.tensor_scalar_max(w2[:], w2[:], 1e-24)
            with tc.high_priority():
                nc.scalar.activation(lg[:], w2[:], AF.Ln)
                nc.scalar.activation(rn[:], lg[:], AF.Exp, scale=-0.5)
            nc.vector.tensor_mul(
                vn[:].rearrange("p (l o) -> p l o", l=L),
                v_u[:].rearrange("p (l o) -> p l o", l=L),
                rn[:].unsqueeze(1).broadcast_to([P, L, O]))
            st["vn"] = vn

        def main(st, it):
            u_sb, vn, S = st["u_sb"], st["vn"], st["S"]
            vn_lo = vn[:].rearrange("p (l o) -> p l o", l=L)
            # DVE z region [0 : ND*256] + tree scratch (tr: max(l-tree h0
            # NH0*160, n-tree L1 28*256) = 7424; tr2: l-tree h1 NH1*160)
            TRW = NH0 * 256  # 7424
            zt = zpool.tile([P, ND * 256 + TRW + NH1 * 160], bf16, tag="z")
            z = zt[:, 0:ND * 256]
            tr = zt[:, ND * 256: ND * 256 + TRW]
            tr2 = zt[:, ND * 256 + TRW:]
            # Pool z region (ptr covers the l-tree's NP*160 = 2400 elems)
            pzt = zpool.tile([P, NP * 256 + 2400 + 512], bf16, tag="pz")
            pz = pzt[:, 0:NP * 256]
            ptr = pzt[:, NP * 256: NP * 256 + 2400]
            ptr2 = pzt[:, NP * 256 + 2400:]

            # logits/e lifetime is within this main call (mains execute
            # in-order per engine), so single-buffered is safe.
            logits = lepool.tile([P, NIN * O], fp32, tag="logits")
            e = lepool.tile([P, NIN * O], bf16, tag="e")

            def zv(ap, lw, nn):
                return ap.rearrange("p (n l o) -> p n l o", n=nn, l=lw)

            def half(eng, zh, uh, trh, lgh, nh):
                # z1 = u * bcast_n(vn); l-tree; logits slice
                eng.tensor_mul(
                    zv(zh, L, nh), uh,
                    vn_lo.unsqueeze(1).broadcast_to([P, nh, L, O]))
                eng.tensor_add(zv(trh[:, 0:nh * 128], 8, nh),
                               zv(zh, L, nh)[:, :, 0:8, :],
                               zv(zh, L, nh)[:, :, 8:16, :])
                eng.tensor_add(zv(zh[:, 0:nh * 64], 4, nh),
                               zv(trh[:, 0:nh * 128], 8, nh)[:, :, 0:4, :],
                               zv(trh[:, 0:nh * 128], 8, nh)[:, :, 4:8, :])
                eng.tensor_add(zv(trh[:, nh * 128:nh * 160], 2, nh),
                               zv(zh[:, 0:nh * 64], 4, nh)[:, :, 0:2, :],
                               zv(zh[:, 0:nh * 64], 4, nh)[:, :, 2:4, :])
                t2 = zv(trh[:, nh * 128:nh * 160], 2, nh)
                eng.tensor_add(lgh.rearrange("p (n o) -> p n o", n=nh),
                               t2[:, :, 0, :], t2[:, :, 1, :])

            def z2(eng, zh, uh, eh, nh):
                eng.tensor_mul(
                    zv(zh, L, nh), uh,
                    eh.rearrange("p (n o) -> p n o", n=nh)
                    .unsqueeze(2).broadcast_to([P, nh, L, O]))

            # ---- DVE slice: two n-halves pipelined through the ACT exp ----
            offs = [(0, NH0), (NH0, NH1)]
            for (n0, nh) in offs:
                zh = z[:, n0 * 256:(n0 + nh) * 256]
                uh = zv(u_sb[:, n0 * 256:(n0 + nh) * 256], L, nh)
                trh = tr if n0 == 0 else tr2
                lgh = logits[:, n0 * O:(n0 + nh) * O]
                half(nc.vector, zh, uh, trh, lgh, nh)
                with tc.high_priority():
                    nc.scalar.activation(e[:, n0 * O:(n0 + nh) * O], lgh,
                                         AF.Exp)
                z2(nc.vector, zh, uh, e[:, n0 * O:(n0 + nh) * O], nh)

            # ---- Pool slice: same chain for n in [ND, NIN) ----
            pzh = pz
            puh = zv(u_sb[:, ND * 256:], L, NP)
            plg = logits[:, ND * O:]
            half(nc.gpsimd, pzh, puh, ptr, plg, NP)
            with tc.high_priority():
                nc.scalar.activation(e[:, ND * O:], plg, AF.Exp)
            z2(nc.gpsimd, pzh, puh, e[:, ND * O:], NP)

            if it == 2:
                nc.vector.reduce_sum(
                    S[:], e[:].rearrange("p (n o) -> p o n", n=NIN),
                    axis=AX.X)

            def add(eng, out, a, b):
                eng.tensor_add(out, a, b)

            # ---- DVE n-tree: 57 -> 28(+1) -> 14(+1c) ... leftovers merged
            # at the end. Block row = 256 elems.
            A = nc.vector
            add(A, tr[:, 0:28 * 256], z[:, 0:28 * 256], z[:, 28 * 256:56 * 256])
            # leftover: z row 56
            add(A, z[:, 0:14 * 256], tr[:, 0:14 * 256], tr[:, 14 * 256:28 * 256])
            add(A, tr[:, 0:7 * 256], z[:, 0:7 * 256], z[:, 7 * 256:14 * 256])
            add(A, z[:, 0:3 * 256], tr[:, 0:3 * 256], tr[:, 3 * 256:6 * 256])
            # leftover: tr row 6
            add(A, tr2[:, 0:256], z[:, 0:256], z[:, 256:512])
            # rows left: tr2[0], z row2, tr row6, z row56
            add(A, z[:, 256:512], z[:, 2 * 256:3 * 256], z[:, 56 * 256:57 * 256])
            add(A, z[:, 0:256], tr2[:, 0:256], tr[:, 6 * 256:7 * 256])
            v_d = small.tile([P, 256], fp32, tag="vd")
            add(A, v_d[:], z[:, 0:256], z[:, 256:512])

            # ---- Pool n-tree: 15 -> 7(+1) -> 3(+1c) -> 1(+1c)
            Pp = nc.gpsimd
            add(Pp, ptr[:, 0:7 * 256], pz[:, 0:7 * 256], pz[:, 7 * 256:14 * 256])
            # leftover pz row 14
            add(Pp, pz[:, 0:3 * 256], ptr[:, 0:3 * 256], ptr[:, 3 * 256:6 * 256])
            # leftover ptr row 6
            add(Pp, ptr2[:, 0:256], pz[:, 0:256], pz[:, 256:512])
            add(Pp, pz[:, 0:256], pz[:, 2 * 256:3 * 256], pz[:, 14 * 256:15 * 256])
            add(Pp, ptr2[:, 256:512], ptr2[:, 0:256], ptr[:, 6 * 256:7 * 256])
            v_p = small.tile([P, 256], fp32, tag="vp")
            add(Pp, v_p[:], pz[:, 0:256], ptr2[:, 256:512])

            # merge
            v_u = small.tile([P, 256], fp32, tag="vu")
            nc.vector.tensor_add(v_u[:], v_d[:], v_p[:])
            st["v_u"] = v_u

        def squash(st, k):
            # ---- squash: out = v_u * sqrt(w2) / (S^2 + w2) ----
            # Pure serial chain at the very end: keep it on DVE+ACT (fewer
            # cross-engine hops than spreading it over Pool).
            v_u, S = st["v_u"], st["S"]
            sq = small.tile([P, 256], fp32, tag="sq")
            nc.vector.tensor_mul(sq[:], v_u[:], v_u[:])
            w2 = small.tile([P, O], fp32, tag="w2")
            nc.vector.reduce_sum(
                w2[:], sq[:].rearrange("p (l o) -> p o l", l=L), axis=AX.X)
            nc.vector.tensor_scalar_max(w2[:], w2[:], 1e-24)
            lg = small.tile([P, O], fp32, tag="lg")
            nc.scalar.activation(lg[:], w2[:], AF.Ln)
            sw = small.tile([P, O], fp32, tag="sw")
            nc.scalar.activation(sw[:], lg[:], AF.Exp, scale=0.5)
            den = small.tile([P, O], fp32, tag="den")
            nc.vector.tensor_mul(den[:], S[:], S[:])
            nc.vector.tensor_add(den[:], den[:], w2[:])
            rden = small.tile([P, O], fp32, tag="rn")
            nc.vector.reciprocal(rden[:], den[:])
            fac = small.tile([P, O], fp32, tag="fac")
            nc.vector.tensor_mul(fac[:], sw[:], rden[:])
            # vfin [p, (o,l)] = v_u viewed (o,l) * bcast_l(fac)
            vfin = small.tile([P, 256], fp32, tag="vfin")
            nc.vector.tensor_mul(
                vfin[:].rearrange("p (o l) -> p o l", o=O),
                v_u[:].rearrange("p (l o) -> p o l", l=L),
                fac[:].unsqueeze(2).broadcast_to([P, O, L]))
            # transpose to channel-major and store
            for half_i in range(2):
                tp = tpsum.tile([128, 128], fp32, tag="tp")
                nc.tensor.transpose(tp[:],
                                    vfin[:, half_i * 128:(half_i + 1) * 128],
                                    ident[:])
                vT = small.tile([128, 128], fp32, tag="vT")
                nc.scalar.copy(vT[:], tp[:])
                nc.sync.dma_start(
                    out=out_d[half_i * 128:(half_i + 1) * 128,
                              4 * k:4 * k + CHUNK_ROWS, :],
                    in_=vT[:].rearrange("f (r w) -> f r w", r=CHUNK_ROWS))

        # Interleave the two chunks' routing iterations: chunk k's small
        # ACT chains (prep/exp) overlap the other chunk's DVE/Pool work.
        GROUPS_P1A = [(s0, s1, 'A') for (s0, s1, _e) in GROUPS_P1]
        GROUPS_P1B = []
        st0 = priors_v0(0)
        priors_u(st0, GROUPS_P1)
        prep(st0)
        priors_u(st0, GROUPS_P2)
        # chunk 1's v0/front drains/prep issued BEFORE chunk 0's first main:
        # vn(c1) must exist by the time Pool rolls off its c0 slice, and the
        # scheduler will not hoist across a whole routing slice on its own.
        # The later c1 drain groups keep their 'D' assignment and are issued
        # after main(st0, 0) so DVE drains them in its post-slice window.
        st1 = priors_v0(1)
        priors_u(st1, GROUPS_P1A)
        prep(st1)
        main(st0, 0)
        prep(st0)
        priors_u(st1, GROUPS_P1B)
        priors_u(st1, GROUPS_P2)
        sts = [st0, st1]
        main(st1, 0)
        prep(st1)
        for k in range(NCHUNK):
            main(sts[k], 1)
            prep(sts[k])
        for k in range(NCHUNK):
            main(sts[k], 2)
        for k in range(NCHUNK):
            squash(sts[k], k)
    nc.compile()
    return nc


_NC_CACHE = {}


def _get_nc():
    if "nc" not in _NC_CACHE:
        _NC_CACHE["nc"] = _build_bass()
    return _NC_CACHE["nc"]


def _shard_inputs(x, weight):
    # wr[m, (t, l, o)] = weight[o, l, m, i, j], t = i*3+j; replicated over
    # g on the host so one DMA fills the K=128 operand.
    wr = np.ascontiguousarray(np.tile(
        weight.transpose(2, 3, 4, 1, 0).reshape(M, NTAP * 256)
        .astype(np.float32), (G, 1)))
    in_maps = []
    for core in range(NCORES):
        b = core // 4
        oh0 = (core % 4) * ROWS_PER_CORE
        xs = np.zeros((CIN, 10, 34), np.float32)
        lo, hi = oh0 - 1, oh0 + 9
        vlo, vhi = max(lo, 0), min(hi, H)
        xs[:, vlo - lo:vhi - lo, 1:33] = x[b, :, vlo:vhi, :]
        # j-shifted channel-major: xs2j[c, j, h, 32]
        xs2j = np.stack([xs[:, :, j:j + 32] for j in range(3)], axis=1)
        xs2j = np.ascontiguousarray(xs2j.reshape(CIN, 3 * 10 * 32))
        # j-shifted m-major: xsj[m, j, g, h, 32]
        xs_m = xs.reshape(G, M, 10, 34)
        xsj = np.stack([xs_m[:, :, :, j:j + 32] for j in range(3)], axis=2)
        # [g, m, j, h, w] -> [m, j, g, h, w]
        xsj = np.ascontiguousarray(
            xsj.transpose(1, 2, 0, 3, 4).reshape(M, 3 * G * 10 * 32))
        in_maps.append({"xs": xsj, "xs2": xs2j, "wgt": wr})
    return in_maps


def _gather_output(results):
    out = np.zeros((B, COUT, H, W), np.float32)
    for core in range(NCORES):
        b = core // 4
        oh0 = (core % 4) * ROWS_PER_CORE
        out[b, :, oh0:oh0 + ROWS_PER_CORE, :] = results[core]["out"]
    return out


def kernel(x: np.ndarray, weight: np.ndarray) -> np.ndarray:
    from concourse.bass_utils import run_bass_kernel_spmd

    x = np.asarray(x, np.float32)
    weight = np.asarray(weight, np.float32)
    res = run_bass_kernel_spmd(_get_nc(), _shard_inputs(x, weight),
                               list(range(NCORES)))
    return _gather_output(res.results)


# revision 55
# speedup vs baseline: 1.0304x; 1.0144x over previous
"""CapsuleConv2d (k-means routing, 3 iters) Trainium2 Bass kernel.

Problem (hardcoded): x [2,128,32,32] f32, weight [16,16,16,3,3] f32
(w[o,l,m,i,j]), stride 1, pad 1, G=8 groups of M=16 in-channels,
N_in = G*KH*KW = 72 votes, O=16 out-capsules of L=16.
Output [2, 256, 32, 32] f32.

Sharding: data-parallel over (b, oh): 64 rows -> 8 cores x 8 rows.
Each core processes 2 chunks of 128 positions (4 oh-rows x 32 ow).

v2 changes over the original baseline (290us):
 - Host supplies 3 column-shifted slabs (one per kw tap j), so every
   3x3-tap window is a contiguous [*, 128] lhsT slice: the Pool/ACT
   patch-assembly copies are gone and matmuls read the slab directly.
 - The Pool engine runs the full routing chain (z1 / l-tree / z2 /
   n-tree) for votes n in [57, 72) concurrently with DVE's n in
   [0, 57): per-iteration wall time drops from ~37.6us (DVE alone at
   2x bf16) to ~31us (DVE ~30us || Pool ~30us), merged by one add.
 - u PSUM->SBUF drains are split across DVE/ACT/Pool at the head
   (chunk 0) and ride on ACT afterwards.
"""
from contextlib import ExitStack

import numpy as np

B, CIN, H, W = 2, 128, 32, 32
G, M, O, L = 8, 16, 16, 16
NTAP, NIN = 9, 72
COUT = O * L
NCORES = 8
ROWS_PER_CORE = 8  # (b, oh) rows per core
CHUNK_ROWS = 4
NCHUNK = ROWS_PER_CORE // CHUNK_ROWS
P = 128
ND = 57            # votes routed on DVE
NP = NIN - ND      # votes routed on Pool (15)
NH0, NH1 = 29, 28  # DVE n-halves for the exp pipeline
ND2, NP2 = 51, 21  # final-iteration split (Pool takes the tail slack)


def _build_bass():
    import concourse.tile as tile
    from concourse import bacc, masks, mybir

    # The act-table pass greedily picks the first set containing each
    # function, ping-ponging exp_and_others <-> natural_log (2.7us/load).
    # Strip Exp/Ln from every set except the combined one so all our ACT
    # work (Exp, Ln, Copy, Identity) lives in a single table set.
    if not getattr(bacc, "_capsule_act_tables_patched", False):
        _orig_gat = bacc.get_activation_tables

        def _gat(arch):
            t = dict(_orig_gat(arch))
            for name, fns in t.items():
                if name != "natural_log_exp_and_others":
                    t[name] = {f for f in fns if f.name not in ("Exp", "Ln")}
            return t

        bacc.get_activation_tables = _gat
        bacc._capsule_act_tables_patched = True

    fp32 = mybir.dt.float32
    f32r = mybir.dt.float32r
    bf16 = mybir.dt.bfloat16
    AX = mybir.AxisListType
    AF = mybir.ActivationFunctionType

    nc = bacc.Bacc("TRN2", target_bir_lowering=False, debug=False)
    # f32r params: same bits as fp32, but lets every DMA queue load them
    # without the gpsimd-only cast path.
    # j-shifted m-major slabs: xs[m, j, g, h(10), w(32)]; window for tap
    # (i, j) rows r..r+3 is contiguous 128 floats -> direct matmul lhsT.
    xs_d = nc.declare_dram_parameter("xs", [M, 3 * G * 10 * 32], f32r,
                                     isOutput=False)
    # j-shifted channel-major slabs: xs2[c=(g,m), j, h(10), w(32)]
    xs2_d = nc.declare_dram_parameter("xs2", [CIN, 3 * 10 * 32], f32r,
                                      isOutput=False)
    # host-pretransposed weights, replicated over g on the host so one DMA
    # fills the full K=128 operand: wr2[(g,m), (t,l,o)] = w[o, l, m, i, j]
    w_d = nc.declare_dram_parameter("wgt", [CIN, NTAP * 256], f32r,
                                    isOutput=False)
    out_d = nc.declare_dram_parameter("out", [COUT, ROWS_PER_CORE, W], fp32,
                                      isOutput=True)

    with tile.TileContext(nc) as tc, ExitStack() as ctx:
        const_pool = ctx.enter_context(tc.tile_pool(name="const", bufs=1))
        upool = ctx.enter_context(tc.tile_pool(name="u", bufs=2))
        zpool = ctx.enter_context(tc.tile_pool(name="z", bufs=1))
        lepool = ctx.enter_context(tc.tile_pool(name="le", bufs=1))
        small = ctx.enter_context(tc.tile_pool(name="small", bufs=2))
        psum = ctx.enter_context(tc.tile_pool(name="ps", bufs=3, space="PSUM"))
        tpsum = ctx.enter_context(tc.tile_pool(name="tps", bufs=1, space="PSUM"))
        vpsum = ctx.enter_context(tc.tile_pool(name="vps", bufs=1, space="PSUM"))

        # ---- constants (once per core) ----
        # Exactly 4 input DMAs (HWDGE descriptor-gen is ~625ns serial per
        # DMA): slab first (gates every u-matmul), then wgt in partition
        # halves (a single 128-partition DMA costs ~3.3us, halves ~0.5us
        # each), slab2 for the K=128 v0.
        wr2 = const_pool.tile([CIN, NTAP * 256], f32r)
        slab_f = const_pool.tile([M, 3 * G * 10 * 32], f32r)
        slab = slab_f[:].rearrange("m (j g h w) -> m j g h w", j=3, g=G, h=10)
        nc.sync.dma_start(out=slab_f[:], in_=xs_d[:])
        nc.scalar.dma_start(out=wr2[0:64, :], in_=w_d[0:64, :])
        slab2_f = const_pool.tile([CIN, 3 * 10 * 32], f32r)
        slab2 = slab2_f[:].rearrange("c (j h w) -> c j h w", j=3, h=10)
        nc.gpsimd.dma_start(out=slab2_f[:], in_=xs2_d[:])
        nc.sync.dma_start(out=wr2[64:128, :], in_=w_d[64:128, :])
        wr_r = wr2[0:M, :]

        ident = const_pool.tile([128, 128], fp32)
        masks.make_identity(nc, ident[:])

        # PE warm-up: ~4us of back-to-back dummy matmuls during the initial
        # DMA wait releases the HAM clock throttle before the real matmuls.
        warm = const_pool.tile([128, 64], bf16)
        nc.vector.memset(warm[:], 0.0)
        wps = tpsum.tile([64, 64], fp32, tag="tp")
        for _ in range(55):
            nc.tensor.matmul(wps[:], warm[:, 0:64], warm[:], start=True,
                             stop=True, tile_position=(0, 0))

        def priors_v0(k):
            # ---- v0 = sum_n u via K=128 matmuls; finishes early so prep
            # (hoisted before the u fill) can compute vn immediately.
            v0_ps = vpsum.tile([P, 256], fp32, tag="v0ps")
            for t in range(NTAP):
                i, j = divmod(t, 3)
                lhsT2 = slab2[:, j, 4 * k + i: 4 * k + i + CHUNK_ROWS, :]
                nc.tensor.matmul(v0_ps[:], lhsT2,
                                 wr2[:, t * 256:(t + 1) * 256],
                                 start=(t == 0), stop=(t == NTAP - 1),
                                 tile_position=(0, 0), skip_group_check=True)
            v_u = small.tile([P, 256], fp32, tag="vu")
            with tc.high_priority():
                nc.scalar.copy(v_u[:], v0_ps[:])
            S = small.tile([P, O], fp32, tag="S")
            u_sb = upool.tile([P, NIN * 256], bf16, tag="u")
            return {"v_u": v_u, "S": S, "vn": None, "k": k, "u_sb": u_sb}

        # u-slot drain groups: Pool's slice [ND, NIN) fills and drains
        # first so the Pool routing chain starts on time, then DVE's h0 and
        # h1. 'D'/'A' pick the drain engine (chunk 0 splits DVE/ACT while
        # DVE is otherwise idle; chunk 1 rides entirely on ACT slack).
        # u-slot drain groups: Pool's slice [ND, NIN) fills and drains
        # first so the Pool routing chain starts on time, then DVE's h0 and
        # h1. 'D'/'A' pick the drain engine; 'D' groups also give DVE
        # useful work while it waits on the vn/u joins.
        GROUPS_P1 = [(57, 61, 'A'), (61, 65, 'D'), (65, 69, 'A'),
                     (69, 72, 'D'),
                     (0, 4, 'A'), (4, 8, 'D'), (8, 12, 'A'), (12, 16, 'D'),
                     (16, 20, 'A'), (20, 24, 'D'), (24, 28, 'A'),
                     (28, 29, 'D')]
        GROUPS_P2 = [(29, 33, 'A'), (33, 37, 'D'), (37, 41, 'A'),
                     (41, 45, 'A'), (45, 49, 'D'), (49, 53, 'A'),
                     (53, 57, 'A')]

        def priors_u(st, groups):
            # ---- priors on PE (fp32r: full-rate, near-fp32 precision).
            # Slot s holds vote (t, g) = divmod(s, 8); routing is symmetric
            # over votes so the assignment is free.
            k = st["k"]
            u_sb = st["u_sb"]
            for (s0, s1, ec) in groups:
                ups = psum.tile([P, 1024], fp32, tag="ups")
                for s in range(s0, s1):
                    t, g = divmod(s, 8)
                    i, j = divmod(t, 3)
                    lhsT = slab[:, j, g, 4 * k + i: 4 * k + i + CHUNK_ROWS, :]
                    nc.tensor.matmul(
                        ups[:, (s - s0) * 256:(s - s0 + 1) * 256],
                        lhsT, wr_r[:, t * 256:(t + 1) * 256],
                        start=True, stop=True, tile_position=(0, 0))
                dst = u_sb[:, s0 * 256:s1 * 256]
                src = ups[:, 0:(s1 - s0) * 256]
                if ec == 'D':
                    nc.vector.tensor_copy(dst, src)
                else:
                    nc.scalar.copy(dst, src)

        def prep(st, hoist=False):
            # vn = v_u * rsqrt(max(sum_l v_u^2, eps)); rsqrt = exp(-ln/2)
            # hoist=True runs the chain at high priority so the scheduler
            # slots it into drain gaps (vn gates BOTH engines' next
            # iteration); used where it would otherwise park behind a
            # whole routing slice.
            v_u = st["v_u"]
            sq = small.tile([P, 256], fp32, tag="sq")
            w2 = small.tile([P, O], fp32, tag="w2")
            lg = small.tile([P, O], fp32, tag="lg")
            rn = small.tile([P, O], fp32, tag="rn")
            vn = small.tile([P, 256], bf16, tag="vn")
            nc.vector.tensor_mul(sq[:], v_u[:], v_u[:])
            nc.vector.reduce_sum(
                w2[:], sq[:].rearrange("p (l o) -> p o l", l=L), axis=AX.X)
            nc.vector.tensor_scalar_max(w2[:], w2[:], 1e-24)
            with tc.high_priority():
                nc.scalar.activation(lg[:], w2[:], AF.Ln)
                nc.scalar.activation(rn[:], lg[:], AF.Exp, scale=-0.5)
            nc.vector.tensor_mul(
                vn[:].rearrange("p (l o) -> p l o", l=L),
                v_u[:].rearrange("p (l o) -> p l o", l=L),
                rn[:].unsqueeze(1).broadcast_to([P, L, O]))
            st["vn"] = vn

        def main(st, it):
            u_sb, vn, S = st["u_sb"], st["vn"], st["S"]
            vn_lo = vn[:].rearrange("p (l o) -> p l o", l=L)
            # DVE z region [0 : ND*256] + tree scratch (tr: max(l-tree h0
            # NH0*160, n-tree L1 28*256) = 7424; tr2: l-tree h1 NH1*160)
            TRW = NH0 * 256  # 7424
            zt = zpool.tile([P, ND * 256 + TRW + NH1 * 160], bf16, tag="z")
            z = zt[:, 0:ND * 256]
            tr = zt[:, ND * 256: ND * 256 + TRW]
            tr2 = zt[:, ND * 256 + TRW:]
            # Pool z region (sized for the NP2 final-iteration slice; ptr
            # covers the l-tree's NP2*160 elems and the n-tree scratch)
            pzt = zpool.tile([P, NP2 * 256 + NP2 * 160], bf16, tag="pz")
            pz = pzt[:, 0:NP2 * 256]
            ptr = pzt[:, NP2 * 256:]

            # logits/e lifetime is within this main call (mains execute
            # in-order per engine), so single-buffered is safe.
            logits = lepool.tile([P, NIN * O], fp32, tag="logits")
            e = lepool.tile([P, NIN * O], bf16, tag="e")

            def zv(ap, lw, nn):
                return ap.rearrange("p (n l o) -> p n l o", n=nn, l=lw)

            def half(eng, zh, uh, trh, lgh, nh):
                # z1 = u * bcast_n(vn); l-tree; logits slice
                eng.tensor_mul(
                    zv(zh, L, nh), uh,
                    vn_lo.unsqueeze(1).broadcast_to([P, nh, L, O]))
                eng.tensor_add(zv(trh[:, 0:nh * 128], 8, nh),
                               zv(zh, L, nh)[:, :, 0:8, :],
                               zv(zh, L, nh)[:, :, 8:16, :])
                eng.tensor_add(zv(zh[:, 0:nh * 64], 4, nh),
                               zv(trh[:, 0:nh * 128], 8, nh)[:, :, 0:4, :],
                               zv(trh[:, 0:nh * 128], 8, nh)[:, :, 4:8, :])
                eng.tensor_add(zv(trh[:, nh * 128:nh * 160], 2, nh),
                               zv(zh[:, 0:nh * 64], 4, nh)[:, :, 0:2, :],
                               zv(zh[:, 0:nh * 64], 4, nh)[:, :, 2:4, :])
                t2 = zv(trh[:, nh * 128:nh * 160], 2, nh)
                eng.tensor_add(lgh.rearrange("p (n o) -> p n o", n=nh),
                               t2[:, :, 0, :], t2[:, :, 1, :])

            def z2(eng, zh, uh, eh, nh):
                eng.tensor_mul(
                    zv(zh, L, nh), uh,
                    eh.rearrange("p (n o) -> p n o", n=nh)
                    .unsqueeze(2).broadcast_to([P, nh, L, O]))

            def sum_rows(eng, Z, TR, n, v_out):
                # Pairwise-halving sum of n contiguous 256-elem rows in Z,
                # ping-ponging with TR (>= n//2 rows). Odd leftovers are
                # collected and folded in at the end (dst rows 5+ of the
                # final `other` buffer are dead data, never leftover rows).
                R = 256
                lefts = []
                cur, other, cn = Z, TR, n
                while cn > 1:
                    h = cn // 2
                    eng.tensor_add(other[:, 0:h * R], cur[:, 0:h * R],
                                   cur[:, h * R:2 * h * R])
                    if cn % 2:
                        lefts.append(cur[:, 2 * h * R:(2 * h + 1) * R])
                    cur, other = other, cur
                    cn = h
                acc = cur[:, 0:R]
                for i, lv in enumerate(lefts):
                    if i == len(lefts) - 1:
                        dst = v_out[:]
                    else:
                        dst = other[:, (5 + i) * R:(6 + i) * R]
                    eng.tensor_add(dst, acc, lv)
                    acc = dst

            # Final iteration shifts 6 votes DVE -> Pool: Pool runs well
            # ahead by then while DVE's last slices are the kernel tail.
            nd, np_ = (ND, NP) if it < 2 else (ND2, NP2)
            h0 = nd - nd // 2

            # ---- DVE slice: two n-halves pipelined through the ACT exp ----
            offs = [(0, h0), (h0, nd - h0)]
            for (n0, nh) in offs:
                zh = z[:, n0 * 256:(n0 + nh) * 256]
                uh = zv(u_sb[:, n0 * 256:(n0 + nh) * 256], L, nh)
                trh = tr if n0 == 0 else tr2
                lgh = logits[:, n0 * O:(n0 + nh) * O]
                half(nc.vector, zh, uh, trh, lgh, nh)
                with tc.high_priority():
                    nc.scalar.activation(e[:, n0 * O:(n0 + nh) * O], lgh,
                                         AF.Exp)
                z2(nc.vector, zh, uh, e[:, n0 * O:(n0 + nh) * O], nh)

            # ---- Pool slice: same chain for n in [nd, NIN) ----
            puh = zv(u_sb[:, nd * 256:], L, np_)
            plg = logits[:, nd * O:]
            half(nc.gpsimd, pz[:, 0:np_ * 256], puh, ptr, plg, np_)
            with tc.high_priority():
                nc.scalar.activation(e[:, nd * O:], plg, AF.Exp)
            z2(nc.gpsimd, pz[:, 0:np_ * 256], puh, e[:, nd * O:], np_)

            if it == 2:
                nc.vector.reduce_sum(
                    S[:], e[:].rearrange("p (n o) -> p o n", n=NIN),
                    axis=AX.X)

            v_d = small.tile([P, 256], fp32, tag="vd")
            sum_rows(nc.vector, z, tr, nd, v_d)
            v_p = small.tile([P, 256], fp32, tag="vp")
            sum_rows(nc.gpsimd, pz, ptr, np_, v_p)

            # merge
            v_u = small.tile([P, 256], fp32, tag="vu")
            nc.vector.tensor_add(v_u[:], v_d[:], v_p[:])
            st["v_u"] = v_u

        def squash(st, k):
            # ---- squash: out = v_u * sqrt(w2) / (S^2 + w2) ----
            # Pure serial chain at the very end: keep it on DVE+ACT (fewer
            # cross-engine hops than spreading it over Pool).
            v_u, S = st["v_u"], st["S"]
            sq = small.tile([P, 256], fp32, tag="sq")
            nc.vector.tensor_mul(sq[:], v_u[:], v_u[:])
            w2 = small.tile([P, O], fp32, tag="w2")
            nc.vector.reduce_sum(
                w2[:], sq[:].rearrange("p (l o) -> p o l", l=L), axis=AX.X)
            nc.vector.tensor_scalar_max(w2[:], w2[:], 1e-24)
            lg = small.tile([P, O], fp32, tag="lg")
            nc.scalar.activation(lg[:], w2[:], AF.Ln)
            sw = small.tile([P, O], fp32, tag="sw")
            nc.scalar.activation(sw[:], lg[:], AF.Exp, scale=0.5)
            den = small.tile([P, O], fp32, tag="den")
            nc.vector.tensor_mul(den[:], S[:], S[:])
            nc.vector.tensor_add(den[:], den[:], w2[:])
            rden = small.tile([P, O], fp32, tag="rn")
            nc.vector.reciprocal(rden[:], den[:])
            fac = small.tile([P, O], fp32, tag="fac")
            nc.vector.tensor_mul(fac[:], sw[:], rden[:])
            # vfin [p, (o,l)] = v_u viewed (o,l) * bcast_l(fac)
            vfin = small.tile([P, 256], fp32, tag="vfin")
            nc.vector.tensor_mul(
                vfin[:].rearrange("p (o l) -> p o l", o=O),
                v_u[:].rearrange("p (l o) -> p o l", l=L),
                fac[:].unsqueeze(2).broadcast_to([P, O, L]))
            # transpose to channel-major and store
            for half_i in range(2):
                tp = tpsum.tile([128, 128], fp32, tag="tp")
                nc.tensor.transpose(tp[:],
                                    vfin[:, half_i * 128:(half_i + 1) * 128],
                                    ident[:])
                vT = small.tile([128, 128], fp32, tag="vT")
                nc.scalar.copy(vT[:], tp[:])
                nc.sync.dma_start(
                    out=out_d[half_i * 128:(half_i + 1) * 128,
                              4 * k:4 * k + CHUNK_ROWS, :],
                    in_=vT[:].rearrange("f (r w) -> f r w", r=CHUNK_ROWS))

        # Interleave the two chunks' routing iterations: chunk k's small
        # ACT chains (prep/exp) overlap the other chunk's DVE/Pool work.
        GROUPS_P1A = [(s0, s1, 'A') for (s0, s1, _e) in GROUPS_P1]
        GROUPS_P1B = []
        st0 = priors_v0(0)
        priors_u(st0, GROUPS_P1)
        prep(st0)
        priors_u(st0, GROUPS_P2)
        # chunk 1's v0/front drains/prep issued BEFORE chunk 0's first main:
        # vn(c1) must exist by the time Pool rolls off its c0 slice, and the
        # scheduler will not hoist across a whole routing slice on its own.
        # The later c1 drain groups keep their 'D' assignment and are issued
        # after main(st0, 0) so DVE drains them in its post-slice window.
        st1 = priors_v0(1)
        priors_u(st1, GROUPS_P1A)
        prep(st1)
        main(st0, 0)
        prep(st0)
        priors_u(st1, GROUPS_P1B)
        priors_u(st1, GROUPS_P2)
        sts = [st0, st1]
        main(st1, 0)
        prep(st1)
        for k in range(NCHUNK):
            main(sts[k], 1)
            prep(sts[k])
        for k in range(NCHUNK):
            main(sts[k], 2)
        for k in range(NCHUNK):
            squash(sts[k], k)
    nc.compile()
    return nc


_NC_CACHE = {}


def _get_nc():
    if "nc" not in _NC_CACHE:
        _NC_CACHE["nc"] = _build_bass()
    return _NC_CACHE["nc"]


def _shard_inputs(x, weight):
    # wr[m, (t, l, o)] = weight[o, l, m, i, j], t = i*3+j; replicated over
    # g on the host so one DMA fills the K=128 operand.
    wr = np.ascontiguousarray(np.tile(
        weight.transpose(2, 3, 4, 1, 0).reshape(M, NTAP * 256)
        .astype(np.float32), (G, 1)))
    in_maps = []
    for core in range(NCORES):
        b = core // 4
        oh0 = (core % 4) * ROWS_PER_CORE
        xs = np.zeros((CIN, 10, 34), np.float32)
        lo, hi = oh0 - 1, oh0 + 9
        vlo, vhi = max(lo, 0), min(hi, H)
        xs[:, vlo - lo:vhi - lo, 1:33] = x[b, :, vlo:vhi, :]
        # j-shifted channel-major: xs2j[c, j, h, 32]
        xs2j = np.stack([xs[:, :, j:j + 32] for j in range(3)], axis=1)
        xs2j = np.ascontiguousarray(xs2j.reshape(CIN, 3 * 10 * 32))
        # j-shifted m-major: xsj[m, j, g, h, 32]
        xs_m = xs.reshape(G, M, 10, 34)
        xsj = np.stack([xs_m[:, :, :, j:j + 32] for j in range(3)], axis=2)
        # [g, m, j, h, w] -> [m, j, g, h, w]
        xsj = np.ascontiguousarray(
            xsj.transpose(1, 2, 0, 3, 4).reshape(M, 3 * G * 10 * 32))
        in_maps.append({"xs": xsj, "xs2": xs2j, "wgt": wr})
    return in_maps


def _gather_output(results):
    out = np.zeros((B, COUT, H, W), np.float32)
    for core in range(NCORES):
        b = core // 4
        oh0 = (core % 4) * ROWS_PER_CORE
        out[b, :, oh0:oh0 + ROWS_PER_CORE, :] = results[core]["out"]
    return out


def kernel(x: np.ndarray, weight: np.ndarray) -> np.ndarray:
    from concourse.bass_utils import run_bass_kernel_spmd

    x = np.asarray(x, np.float32)
    weight = np.asarray(weight, np.float32)
    res = run_bass_kernel_spmd(_get_nc(), _shard_inputs(x, weight),
                               list(range(NCORES)))
    return _gather_output(res.results)


# revision 60
# speedup vs baseline: 1.0372x; 1.0066x over previous
"""CapsuleConv2d (k-means routing, 3 iters) Trainium2 Bass kernel.

Problem (hardcoded): x [2,128,32,32] f32, weight [16,16,16,3,3] f32
(w[o,l,m,i,j]), stride 1, pad 1, G=8 groups of M=16 in-channels,
N_in = G*KH*KW = 72 votes, O=16 out-capsules of L=16.
Output [2, 256, 32, 32] f32.

Sharding: data-parallel over (b, oh): 64 rows -> 8 cores x 8 rows.
Each core processes 2 chunks of 128 positions (4 oh-rows x 32 ow).

v2 changes over the original baseline (290us):
 - Host supplies 3 column-shifted slabs (one per kw tap j), so every
   3x3-tap window is a contiguous [*, 128] lhsT slice: the Pool/ACT
   patch-assembly copies are gone and matmuls read the slab directly.
 - The Pool engine runs the full routing chain (z1 / l-tree / z2 /
   n-tree) for votes n in [57, 72) concurrently with DVE's n in
   [0, 57): per-iteration wall time drops from ~37.6us (DVE alone at
   2x bf16) to ~31us (DVE ~30us || Pool ~30us), merged by one add.
 - u PSUM->SBUF drains are split across DVE/ACT/Pool at the head
   (chunk 0) and ride on ACT afterwards.
"""
from contextlib import ExitStack

import numpy as np

B, CIN, H, W = 2, 128, 32, 32
G, M, O, L = 8, 16, 16, 16
NTAP, NIN = 9, 72
COUT = O * L
NCORES = 8
ROWS_PER_CORE = 8  # (b, oh) rows per core
CHUNK_ROWS = 4
NCHUNK = ROWS_PER_CORE // CHUNK_ROWS
P = 128
ND = 57            # votes routed on DVE
NP = NIN - ND      # votes routed on Pool (15)
NH0, NH1 = 29, 28  # DVE n-halves for the exp pipeline
ND2, NP2 = 52, 20  # final-iteration split (Pool takes the tail slack)


def _build_bass():
    import concourse.tile as tile
    from concourse import bacc, masks, mybir

    # The act-table pass greedily picks the first set containing each
    # function, ping-ponging exp_and_others <-> natural_log (2.7us/load).
    # Strip Exp/Ln from every set except the combined one so all our ACT
    # work (Exp, Ln, Copy, Identity) lives in a single table set.
    if not getattr(bacc, "_capsule_act_tables_patched", False):
        _orig_gat = bacc.get_activation_tables

        def _gat(arch):
            t = dict(_orig_gat(arch))
            for name, fns in t.items():
                if name != "natural_log_exp_and_others":
                    t[name] = {f for f in fns if f.name not in ("Exp", "Ln")}
            return t

        bacc.get_activation_tables = _gat
        bacc._capsule_act_tables_patched = True

    fp32 = mybir.dt.float32
    f32r = mybir.dt.float32r
    bf16 = mybir.dt.bfloat16
    AX = mybir.AxisListType
    AF = mybir.ActivationFunctionType

    nc = bacc.Bacc("TRN2", target_bir_lowering=False, debug=False)
    # f32r params: same bits as fp32, but lets every DMA queue load them
    # without the gpsimd-only cast path.
    # j-shifted m-major slabs: xs[m, j, g, h(10), w(32)]; window for tap
    # (i, j) rows r..r+3 is contiguous 128 floats -> direct matmul lhsT.
    xs_d = nc.declare_dram_parameter("xs", [M, 3 * G * 10 * 32], f32r,
                                     isOutput=False)
    # j-shifted channel-major slabs: xs2[c=(g,m), j, h(10), w(32)]
    xs2_d = nc.declare_dram_parameter("xs2", [CIN, 3 * 10 * 32], f32r,
                                      isOutput=False)
    # host-pretransposed weights, replicated over g on the host so one DMA
    # fills the full K=128 operand: wr2[(g,m), (t,l,o)] = w[o, l, m, i, j]
    w_d = nc.declare_dram_parameter("wgt", [CIN, NTAP * 256], f32r,
                                    isOutput=False)
    out_d = nc.declare_dram_parameter("out", [COUT, ROWS_PER_CORE, W], fp32,
                                      isOutput=True)

    with tile.TileContext(nc) as tc, ExitStack() as ctx:
        const_pool = ctx.enter_context(tc.tile_pool(name="const", bufs=1))
        upool = ctx.enter_context(tc.tile_pool(name="u", bufs=2))
        zpool = ctx.enter_context(tc.tile_pool(name="z", bufs=1))
        lepool = ctx.enter_context(tc.tile_pool(name="le", bufs=1))
        small = ctx.enter_context(tc.tile_pool(name="small", bufs=2))
        psum = ctx.enter_context(tc.tile_pool(name="ps", bufs=3, space="PSUM"))
        tpsum = ctx.enter_context(tc.tile_pool(name="tps", bufs=1, space="PSUM"))
        vpsum = ctx.enter_context(tc.tile_pool(name="vps", bufs=1, space="PSUM"))

        # ---- constants (once per core) ----
        # Exactly 4 input DMAs (HWDGE descriptor-gen is ~625ns serial per
        # DMA): slab first (gates every u-matmul), then wgt in partition
        # halves (a single 128-partition DMA costs ~3.3us, halves ~0.5us
        # each), slab2 for the K=128 v0.
        wr2 = const_pool.tile([CIN, NTAP * 256], f32r)
        slab_f = const_pool.tile([M, 3 * G * 10 * 32], f32r)
        slab = slab_f[:].rearrange("m (j g h w) -> m j g h w", j=3, g=G, h=10)
        nc.sync.dma_start(out=slab_f[:], in_=xs_d[:])
        nc.scalar.dma_start(out=wr2[0:64, :], in_=w_d[0:64, :])
        slab2_f = const_pool.tile([CIN, 3 * 10 * 32], f32r)
        slab2 = slab2_f[:].rearrange("c (j h w) -> c j h w", j=3, h=10)
        nc.gpsimd.dma_start(out=slab2_f[:], in_=xs2_d[:])
        nc.sync.dma_start(out=wr2[64:128, :], in_=w_d[64:128, :])
        wr_r = wr2[0:M, :]

        ident = const_pool.tile([128, 128], fp32)
        masks.make_identity(nc, ident[:])

        # PE warm-up: ~4us of back-to-back dummy matmuls during the initial
        # DMA wait releases the HAM clock throttle before the real matmuls.
        warm = const_pool.tile([128, 64], bf16)
        nc.vector.memset(warm[:], 0.0)
        wps = tpsum.tile([64, 64], fp32, tag="tp")
        for _ in range(55):
            nc.tensor.matmul(wps[:], warm[:, 0:64], warm[:], start=True,
                             stop=True, tile_position=(0, 0))

        def priors_v0(k):
            # ---- v0 = sum_n u via K=128 matmuls; finishes early so prep
            # (hoisted before the u fill) can compute vn immediately.
            v0_ps = vpsum.tile([P, 256], fp32, tag="v0ps")
            for t in range(NTAP):
                i, j = divmod(t, 3)
                lhsT2 = slab2[:, j, 4 * k + i: 4 * k + i + CHUNK_ROWS, :]
                nc.tensor.matmul(v0_ps[:], lhsT2,
                                 wr2[:, t * 256:(t + 1) * 256],
                                 start=(t == 0), stop=(t == NTAP - 1),
                                 tile_position=(0, 0), skip_group_check=True)
            v_u = small.tile([P, 256], fp32, tag="vu")
            with tc.high_priority():
                nc.scalar.copy(v_u[:], v0_ps[:])
            S = small.tile([P, O], fp32, tag="S")
            u_sb = upool.tile([P, NIN * 256], bf16, tag="u")
            return {"v_u": v_u, "S": S, "vn": None, "k": k, "u_sb": u_sb}

        # u-slot drain groups: Pool's slice [ND, NIN) fills and drains
        # first so the Pool routing chain starts on time, then DVE's h0 and
        # h1. 'D'/'A' pick the drain engine (chunk 0 splits DVE/ACT while
        # DVE is otherwise idle; chunk 1 rides entirely on ACT slack).
        # u-slot drain groups: Pool's slice [ND, NIN) fills and drains
        # first so the Pool routing chain starts on time, then DVE's h0 and
        # h1. 'D'/'A' pick the drain engine; 'D' groups also give DVE
        # useful work while it waits on the vn/u joins.
        GROUPS_P1 = [(57, 61, 'A'), (61, 65, 'D'), (65, 69, 'A'),
                     (69, 72, 'D'),
                     (0, 4, 'A'), (4, 8, 'D'), (8, 12, 'A'), (12, 16, 'D'),
                     (16, 20, 'A'), (20, 24, 'D'), (24, 28, 'A'),
                     (28, 29, 'D')]
        GROUPS_P2 = [(29, 33, 'A'), (33, 37, 'D'), (37, 41, 'A'),
                     (41, 45, 'A'), (45, 49, 'D'), (49, 53, 'A'),
                     (53, 57, 'A')]

        def priors_u(st, groups):
            # ---- priors on PE (fp32r: full-rate, near-fp32 precision).
            # Slot s holds vote (t, g) = divmod(s, 8); routing is symmetric
            # over votes so the assignment is free.
            k = st["k"]
            u_sb = st["u_sb"]
            for (s0, s1, ec) in groups:
                ups = psum.tile([P, 1024], fp32, tag="ups")
                for s in range(s0, s1):
                    t, g = divmod(s, 8)
                    i, j = divmod(t, 3)
                    lhsT = slab[:, j, g, 4 * k + i: 4 * k + i + CHUNK_ROWS, :]
                    nc.tensor.matmul(
                        ups[:, (s - s0) * 256:(s - s0 + 1) * 256],
                        lhsT, wr_r[:, t * 256:(t + 1) * 256],
                        start=True, stop=True, tile_position=(0, 0))
                dst = u_sb[:, s0 * 256:s1 * 256]
                src = ups[:, 0:(s1 - s0) * 256]
                if ec == 'D':
                    nc.vector.tensor_copy(dst, src)
                else:
                    nc.scalar.copy(dst, src)

        def prep(st, hoist=False):
            # vn = v_u * rsqrt(max(sum_l v_u^2, eps)); rsqrt = exp(-ln/2)
            # hoist=True runs the chain at high priority so the scheduler
            # slots it into drain gaps (vn gates BOTH engines' next
            # iteration); used where it would otherwise park behind a
            # whole routing slice.
            v_u = st["v_u"]
            sq = small.tile([P, 256], fp32, tag="sq")
            w2 = small.tile([P, O], fp32, tag="w2")
            lg = small.tile([P, O], fp32, tag="lg")
            rn = small.tile([P, O], fp32, tag="rn")
            vn = small.tile([P, 256], bf16, tag="vn")
            nc.vector.tensor_mul(sq[:], v_u[:], v_u[:])
            nc.vector.reduce_sum(
                w2[:], sq[:].rearrange("p (l o) -> p o l", l=L), axis=AX.X)
            nc.vector.tensor_scalar_max(w2[:], w2[:], 1e-24)
            with tc.high_priority():
                nc.scalar.activation(lg[:], w2[:], AF.Ln)
                nc.scalar.activation(rn[:], lg[:], AF.Exp, scale=-0.5)
            nc.vector.tensor_mul(
                vn[:].rearrange("p (l o) -> p l o", l=L),
                v_u[:].rearrange("p (l o) -> p l o", l=L),
                rn[:].unsqueeze(1).broadcast_to([P, L, O]))
            st["vn"] = vn

        def main(st, it):
            u_sb, vn, S = st["u_sb"], st["vn"], st["S"]
            vn_lo = vn[:].rearrange("p (l o) -> p l o", l=L)
            # DVE z region [0 : ND*256] + tree scratch (tr: max(l-tree h0
            # NH0*160, n-tree L1 28*256) = 7424; tr2: l-tree h1 NH1*160)
            TRW = NH0 * 256  # 7424
            zt = zpool.tile([P, ND * 256 + TRW + NH1 * 160], bf16, tag="z")
            z = zt[:, 0:ND * 256]
            tr = zt[:, ND * 256: ND * 256 + TRW]
            tr2 = zt[:, ND * 256 + TRW:]
            # Pool z region (sized for the NP2 final-iteration slice; ptr
            # covers the l-tree's NP2*160 elems and the n-tree scratch)
            pzt = zpool.tile([P, NP2 * 256 + NP2 * 160], bf16, tag="pz")
            pz = pzt[:, 0:NP2 * 256]
            ptr = pzt[:, NP2 * 256:]

            # logits/e lifetime is within this main call (mains execute
            # in-order per engine), so single-buffered is safe.
            logits = lepool.tile([P, NIN * O], fp32, tag="logits")
            e = lepool.tile([P, NIN * O], bf16, tag="e")

            def zv(ap, lw, nn):
                return ap.rearrange("p (n l o) -> p n l o", n=nn, l=lw)

            def half(eng, zh, uh, trh, lgh, nh):
                # z1 = u * bcast_n(vn); l-tree; logits slice
                eng.tensor_mul(
                    zv(zh, L, nh), uh,
                    vn_lo.unsqueeze(1).broadcast_to([P, nh, L, O]))
                eng.tensor_add(zv(trh[:, 0:nh * 128], 8, nh),
                               zv(zh, L, nh)[:, :, 0:8, :],
                               zv(zh, L, nh)[:, :, 8:16, :])
                eng.tensor_add(zv(zh[:, 0:nh * 64], 4, nh),
                               zv(trh[:, 0:nh * 128], 8, nh)[:, :, 0:4, :],
                               zv(trh[:, 0:nh * 128], 8, nh)[:, :, 4:8, :])
                eng.tensor_add(zv(trh[:, nh * 128:nh * 160], 2, nh),
                               zv(zh[:, 0:nh * 64], 4, nh)[:, :, 0:2, :],
                               zv(zh[:, 0:nh * 64], 4, nh)[:, :, 2:4, :])
                t2 = zv(trh[:, nh * 128:nh * 160], 2, nh)
                eng.tensor_add(lgh.rearrange("p (n o) -> p n o", n=nh),
                               t2[:, :, 0, :], t2[:, :, 1, :])

            def z2(eng, zh, uh, eh, nh):
                eng.tensor_mul(
                    zv(zh, L, nh), uh,
                    eh.rearrange("p (n o) -> p n o", n=nh)
                    .unsqueeze(2).broadcast_to([P, nh, L, O]))

            def sum_rows(eng, Z, TR, n, v_out):
                # Pairwise-halving sum of n contiguous 256-elem rows in Z,
                # ping-ponging with TR (>= n//2 rows). Odd leftovers are
                # collected and folded in at the end (dst rows 5+ of the
                # final `other` buffer are dead data, never leftover rows).
                R = 256
                lefts = []
                cur, other, cn = Z, TR, n
                while cn > 1:
                    h = cn // 2
                    eng.tensor_add(other[:, 0:h * R], cur[:, 0:h * R],
                                   cur[:, h * R:2 * h * R])
                    if cn % 2:
                        lefts.append(cur[:, 2 * h * R:(2 * h + 1) * R])
                    cur, other = other, cur
                    cn = h
                acc = cur[:, 0:R]
                for i, lv in enumerate(lefts):
                    if i == len(lefts) - 1:
                        dst = v_out[:]
                    else:
                        dst = other[:, (5 + i) * R:(6 + i) * R]
                    eng.tensor_add(dst, acc, lv)
                    acc = dst

            # Final iteration shifts 6 votes DVE -> Pool: Pool runs well
            # ahead by then while DVE's last slices are the kernel tail.
            nd, np_ = (ND, NP) if it < 2 else (ND2, NP2)
            h0 = nd - nd // 2

            # ---- DVE slice: two n-halves pipelined through the ACT exp ----
            offs = [(0, h0), (h0, nd - h0)]
            for (n0, nh) in offs:
                zh = z[:, n0 * 256:(n0 + nh) * 256]
                uh = zv(u_sb[:, n0 * 256:(n0 + nh) * 256], L, nh)
                trh = tr if n0 == 0 else tr2
                lgh = logits[:, n0 * O:(n0 + nh) * O]
                half(nc.vector, zh, uh, trh, lgh, nh)
                with tc.high_priority():
                    nc.scalar.activation(e[:, n0 * O:(n0 + nh) * O], lgh,
                                         AF.Exp)
                z2(nc.vector, zh, uh, e[:, n0 * O:(n0 + nh) * O], nh)

            # ---- Pool slice: same chain for n in [nd, NIN) ----
            puh = zv(u_sb[:, nd * 256:], L, np_)
            plg = logits[:, nd * O:]
            half(nc.gpsimd, pz[:, 0:np_ * 256], puh, ptr, plg, np_)
            with tc.high_priority():
                nc.scalar.activation(e[:, nd * O:], plg, AF.Exp)
            z2(nc.gpsimd, pz[:, 0:np_ * 256], puh, e[:, nd * O:], np_)

            if it == 2:
                nc.vector.reduce_sum(
                    S[:], e[:].rearrange("p (n o) -> p o n", n=NIN),
                    axis=AX.X)

            v_d = small.tile([P, 256], fp32, tag="vd")
            sum_rows(nc.vector, z, tr, nd, v_d)
            v_p = small.tile([P, 256], fp32, tag="vp")
            sum_rows(nc.gpsimd, pz, ptr, np_, v_p)

            # merge
            v_u = small.tile([P, 256], fp32, tag="vu")
            nc.vector.tensor_add(v_u[:], v_d[:], v_p[:])
            st["v_u"] = v_u

        def squash(st, k):
            # ---- squash: out = v_u * sqrt(w2) / (S^2 + w2) ----
            # Pure serial chain at the very end: keep it on DVE+ACT (fewer
            # cross-engine hops than spreading it over Pool).
            v_u, S = st["v_u"], st["S"]
            sq = small.tile([P, 256], fp32, tag="sq")
            nc.vector.tensor_mul(sq[:], v_u[:], v_u[:])
            w2 = small.tile([P, O], fp32, tag="w2")
            nc.vector.reduce_sum(
                w2[:], sq[:].rearrange("p (l o) -> p o l", l=L), axis=AX.X)
            nc.vector.tensor_scalar_max(w2[:], w2[:], 1e-24)
            lg = small.tile([P, O], fp32, tag="lg")
            nc.scalar.activation(lg[:], w2[:], AF.Ln)
            sw = small.tile([P, O], fp32, tag="sw")
            nc.scalar.activation(sw[:], lg[:], AF.Exp, scale=0.5)
            den = small.tile([P, O], fp32, tag="den")
            nc.vector.tensor_mul(den[:], S[:], S[:])
            nc.vector.tensor_add(den[:], den[:], w2[:])
            rden = small.tile([P, O], fp32, tag="rn")
            nc.vector.reciprocal(rden[:], den[:])
            fac = small.tile([P, O], fp32, tag="fac")
            nc.vector.tensor_mul(fac[:], sw[:], rden[:])
            # vfin [p, (o,l)] = v_u viewed (o,l) * bcast_l(fac)
            vfin = small.tile([P, 256], fp32, tag="vfin")
            nc.vector.tensor_mul(
                vfin[:].rearrange("p (o l) -> p o l", o=O),
                v_u[:].rearrange("p (l o) -> p o l", l=L),
                fac[:].unsqueeze(2).broadcast_to([P, O, L]))
            # transpose to channel-major and store
            for half_i in range(2):
                tp = tpsum.tile([128, 128], fp32, tag="tp")
                nc.tensor.transpose(tp[:],
                                    vfin[:, half_i * 128:(half_i + 1) * 128],
                                    ident[:])
                vT = small.tile([128, 128], fp32, tag="vT")
                nc.scalar.copy(vT[:], tp[:])
                nc.sync.dma_start(
                    out=out_d[half_i * 128:(half_i + 1) * 128,
                              4 * k:4 * k + CHUNK_ROWS, :],
                    in_=vT[:].rearrange("f (r w) -> f r w", r=CHUNK_ROWS))

        # Interleave the two chunks' routing iterations: chunk k's small
        # ACT chains (prep/exp) overlap the other chunk's DVE/Pool work.
        GROUPS_P1A = [(s0, s1, 'A') for (s0, s1, _e) in GROUPS_P1]
        GROUPS_P1B = []
        st0 = priors_v0(0)
        priors_u(st0, GROUPS_P1)
        prep(st0)
        priors_u(st0, GROUPS_P2)
        # chunk 1's v0/front drains/prep issued BEFORE chunk 0's first main:
        # vn(c1) must exist by the time Pool rolls off its c0 slice, and the
        # scheduler will not hoist across a whole routing slice on its own.
        # The later c1 drain groups keep their 'D' assignment and are issued
        # after main(st0, 0) so DVE drains them in its post-slice window.
        st1 = priors_v0(1)
        priors_u(st1, GROUPS_P1A)
        prep(st1)
        main(st0, 0)
        prep(st0)
        priors_u(st1, GROUPS_P1B)
        priors_u(st1, GROUPS_P2)
        sts = [st0, st1]
        main(st1, 0)
        prep(st1)
        for k in range(NCHUNK):
            main(sts[k], 1)
            prep(sts[k])
        for k in range(NCHUNK):
            main(sts[k], 2)
        for k in range(NCHUNK):
            squash(sts[k], k)
    nc.compile()
    return nc


_NC_CACHE = {}


def _get_nc():
    if "nc" not in _NC_CACHE:
        _NC_CACHE["nc"] = _build_bass()
    return _NC_CACHE["nc"]


def _shard_inputs(x, weight):
    # wr[m, (t, l, o)] = weight[o, l, m, i, j], t = i*3+j; replicated over
    # g on the host so one DMA fills the K=128 operand.
    wr = np.ascontiguousarray(np.tile(
        weight.transpose(2, 3, 4, 1, 0).reshape(M, NTAP * 256)
        .astype(np.float32), (G, 1)))
    in_maps = []
    for core in range(NCORES):
        b = core // 4
        oh0 = (core % 4) * ROWS_PER_CORE
        xs = np.zeros((CIN, 10, 34), np.float32)
        lo, hi = oh0 - 1, oh0 + 9
        vlo, vhi = max(lo, 0), min(hi, H)
        xs[:, vlo - lo:vhi - lo, 1:33] = x[b, :, vlo:vhi, :]
        # j-shifted channel-major: xs2j[c, j, h, 32]
        xs2j = np.stack([xs[:, :, j:j + 32] for j in range(3)], axis=1)
        xs2j = np.ascontiguousarray(xs2j.reshape(CIN, 3 * 10 * 32))
        # j-shifted m-major: xsj[m, j, g, h, 32]
        xs_m = xs.reshape(G, M, 10, 34)
        xsj = np.stack([xs_m[:, :, :, j:j + 32] for j in range(3)], axis=2)
        # [g, m, j, h, w] -> [m, j, g, h, w]
        xsj = np.ascontiguousarray(
            xsj.transpose(1, 2, 0, 3, 4).reshape(M, 3 * G * 10 * 32))
        in_maps.append({"xs": xsj, "xs2": xs2j, "wgt": wr})
    return in_maps


def _gather_output(results):
    out = np.zeros((B, COUT, H, W), np.float32)
    for core in range(NCORES):
        b = core // 4
        oh0 = (core % 4) * ROWS_PER_CORE
        out[b, :, oh0:oh0 + ROWS_PER_CORE, :] = results[core]["out"]
    return out


def kernel(x: np.ndarray, weight: np.ndarray) -> np.ndarray:
    from concourse.bass_utils import run_bass_kernel_spmd

    x = np.asarray(x, np.float32)
    weight = np.asarray(weight, np.float32)
    res = run_bass_kernel_spmd(_get_nc(), _shard_inputs(x, weight),
                               list(range(NCORES)))
    return _gather_output(res.results)
